# revision 1
# baseline (speedup 1.0000x reference)
"""Trainium2 Bass kernel for KroneckerLinear: y = x @ kron(U, V).

Math: with x[t] reshaped to X_t [i1=128, i2=128] (i2 contiguous) and
y[t] reshaped to Y_t [j1=128, j2=128] (j2 contiguous):

    Y_t = U^T @ X_t @ V

Both stages map onto the PE array with the *token* tile as the stationary
operand (lhsT), so every tensor stays in its natural (contiguous) layout
and no transposes are needed anywhere:

    MM1: out = lhsT.T @ rhs with lhsT = X_t  [i1, i2], rhs = U [i1, j1]
         -> P^T [i2, j1]   (P = U^T X_t)
    MM2: lhsT = P^T [i2, j1], rhs = V [i2, j2]
         -> Y_t [j1, j2]

Sharding: data-parallel over the token dim, 256 tokens per core x 8 cores.
"""

import sys

if "/opt/trn_rl_repo" not in sys.path:
    sys.path.insert(0, "/opt/trn_rl_repo")

import numpy as np

import concourse.bacc as bacc
import concourse.bass as bass
import concourse.mybir as mybir
from concourse import tile
from concourse.bass_utils import run_bass_kernel_spmd

F32 = mybir.dt.float32
F32R = mybir.dt.float32r

N_CORES = 8
TOKENS = 2048
D = 16384  # 128 * 128
T_CORE = TOKENS // N_CORES  # 256


def build_nc(n_tokens=T_CORE, mode="fp32r", group=32, quad=4, lgran=0, sgran=0):
    """Build + compile the per-core program.

    mode:
      "fp32"   - exact fp32 matmuls (4 cycles/row on PE)
      "fp32r"  - float32r matmuls with the moving operand padded to 256
                 columns ([U|U], [V|V]) to hit the 1 cycle/row fast path.
                 fp32r operands must be *produced* rounded: x tiles are
                 cast during the (SWDGE) load, P tiles by the DVE copy.
      "fp32rh" - like fp32r but x/U/V are pre-rounded to the fp32r grid
                 on the host (zeroed low 12 mantissa bits), declared as
                 float32r DRAM tensors, and loaded with plain HWDGE DMA.

    lgran/sgran: tokens per load/store dma_start (0 = whole group in one
    strided transfer). gran=1 gives fully contiguous 64 KB transfers,
    which keeps the HBM access stream sequential.
    """
    assert n_tokens % group == 0 and group % quad == 0
    r = mode in ("fp32r", "fp32rh")
    hostround = mode == "fp32rh"
    rwide = 256 if r else 128
    mmdt = F32R if r else F32
    dramdt = F32R if hostround else F32
    lgran = lgran or group
    sgran = sgran or group
    assert group % lgran == 0 and group % sgran == 0

    nc = bacc.Bacc("TRN2", target_bir_lowering=False, debug=False)
    x = nc.dram_tensor("x", [n_tokens, D], dramdt, kind="ExternalInput")
    u = nc.dram_tensor("u", [128, rwide], dramdt, kind="ExternalInput")
    v = nc.dram_tensor("v", [128, rwide], dramdt, kind="ExternalInput")
    y = nc.dram_tensor("y", [n_tokens, D], F32, kind="ExternalOutput")

    with tile.TileContext(nc) as tc:
        with (
            tc.tile_pool(name="const", bufs=1) as cpool,
            tc.tile_pool(name="xin", bufs=2) as xpool,
            tc.tile_pool(name="yout", bufs=2) as ypool,
            tc.tile_pool(name="pmid", bufs=4) as ppool,
            tc.tile_pool(name="ps", bufs=2, space="PSUM") as pspool,
        ):
            u_sb = cpool.tile([128, rwide], mmdt)
            v_sb = cpool.tile([128, rwide], mmdt)
            ld_const = (
                nc.gpsimd.dma_start if (r and not hostround) else nc.sync.dma_start
            )
            ld_const(u_sb[:], u[:])
            ld_const(v_sb[:], v[:])

            for g in range(n_tokens // group):
                xt = xpool.tile([128, group, 128], mmdt)
                ld_x = (
                    nc.gpsimd.dma_start if (r and not hostround) else nc.sync.dma_start
                )
                for c in range(group // lgran):
                    t0 = g * group + c * lgran
                    ld_x(
                        xt[:, c * lgran : (c + 1) * lgran, :],
                        x[t0 : t0 + lgran].rearrange("t (i1 i2) -> i1 t i2", i1=128),
                    )
                yt = ypool.tile([128, group, 128], F32)
                for q in range(group // quad):
                    pa = pspool.tile([128, quad, rwide], F32)
                    for j in range(quad):
                        nc.tensor.matmul(
                            pa[:, j, :],
                            lhsT=xt[:, q * quad + j, :],
                            rhs=u_sb[:],
                            start=True,
                            stop=True,
                        )
                    psb = ppool.tile([128, quad, 128], mmdt)
                    nc.vector.tensor_copy(psb[:], pa[:, :, 0:128])
                    pb = pspool.tile([128, quad, rwide], F32)
                    for j in range(quad):
                        nc.tensor.matmul(
                            pb[:, j, :],
                            lhsT=psb[:, j, :],
                            rhs=v_sb[:],
                            start=True,
                            stop=True,
                        )
                    nc.vector.tensor_copy(
                        yt[:, q * quad : (q + 1) * quad, :], pb[:, :, 0:128]
                    )
                for c in range(group // sgran):
                    t0 = g * group + c * sgran
                    nc.scalar.dma_start(
                        y[t0 : t0 + sgran].rearrange("t (j1 j2) -> j1 t j2", j1=128),
                        yt[:, c * sgran : (c + 1) * sgran, :],
                    )
    nc.compile()
    return nc


_NC_CACHE = {}


def _get_nc(n_tokens, mode, group, quad, lgran, sgran):
    key = (n_tokens, mode, group, quad, lgran, sgran)
    if key not in _NC_CACHE:
        _NC_CACHE[key] = build_nc(n_tokens, mode, group, quad, lgran, sgran)
    return _NC_CACHE[key]


def round_fp32r(a):
    """Round fp32 array to the fp32r grid (11-bit mantissa, round-to-nearest)."""
    u = np.ascontiguousarray(a, dtype=np.float32).view(np.uint32)
    r = ((u + np.uint32(0x800)) & np.uint32(0xFFFFF000)).view(np.float32)
    return np.where(np.isfinite(a), r, a).astype(np.float32)


def _prep_inputs(x, U, V, mode):
    x = np.ascontiguousarray(np.asarray(x), dtype=np.float32)
    U = np.ascontiguousarray(np.asarray(U), dtype=np.float32)
    V = np.ascontiguousarray(np.asarray(V), dtype=np.float32)
    if mode in ("fp32r", "fp32rh"):
        U = np.concatenate([U, U], axis=1)
        V = np.concatenate([V, V], axis=1)
    if mode == "fp32rh":
        x = round_fp32r(x)
        U = round_fp32r(U)
        V = round_fp32r(V)
    return x, U, V


def run(x, U, V, mode="fp32rh", group=32, quad=4, lgran=0, sgran=0,
        trace=False, **spmd_kwargs):
    """Shard over 8 cores, run, gather. Returns (y_full, BassKernelResults)."""
    x, U, V = _prep_inputs(x, U, V, mode)
    t_core = x.shape[0] // N_CORES
    nc = _get_nc(t_core, mode, group, quad, lgran, sgran)
    in_maps = [
        {"x": x[i * t_core : (i + 1) * t_core], "u": U, "v": V}
        for i in range(N_CORES)
    ]
    res = run_bass_kernel_spmd(
        nc, in_maps, list(range(N_CORES)), trace=trace, **spmd_kwargs
    )
    out = np.concatenate([res.results[i]["y"] for i in range(N_CORES)], axis=0)
    return out, res


def kernel(x, U, V):
    out, _ = run(x, U, V)
    return out



# revision 2
# speedup vs baseline: 2.0976x; 2.0976x over previous
"""Trainium2 Bass kernel for KroneckerLinear: y = x @ kron(U, V).

Math: with x[t] reshaped to X_t [i1=128, i2=128] (i2 contiguous) and
y[t] reshaped to Y_t [j1=128, j2=128] (j2 contiguous):

    Y_t = U^T @ X_t @ V

Dataflow (all bf16 on the wire, fp32 accumulation in PSUM):

  stage 1 (per token, token tile stationary):
      MM: out = lhsT.T @ rhs, lhsT = X_t [i1, i2], rhs = U [i1, j1]
          -> P_t = X_t^T U  laid out [i2, j1] in PSUM.
      Four tokens share one PSUM bank: pa = [i2, (t4, j1)].
  stage 2 (batched, V stationary, loaded once):
      MM: lhsT = V [i2, j2], rhs = P [i2, (t4, j1)] (bf16 copy of pa)
          -> Y [j2, (t4, j1)] in PSUM, one N=512 matmul per 4 tokens.

Layouts are chosen so every DMA is a contiguous per-partition run:
  x is pre-swizzled on the host to [i1, t, i2] (per core), y leaves the
  device as [j2, t, j1] and is unscrambled on the host. With group=32
  tokens per DMA that is 8 KB per partition per transfer.

Sharding: data-parallel over tokens, 256 tokens per core x 8 cores.
Host converts x/U/V to bf16 (rel err ~5e-3 vs fp32 reference) and
upcasts the bf16 y back to fp32.
"""

import sys

if "/opt/trn_rl_repo" not in sys.path:
    sys.path.insert(0, "/opt/trn_rl_repo")

import ml_dtypes
import numpy as np

import concourse.bacc as bacc
import concourse.bass as bass
import concourse.mybir as mybir
from concourse import tile
from concourse.bass_utils import run_bass_kernel_spmd

F32 = mybir.dt.float32
BF16 = mybir.dt.bfloat16
NP_BF16 = ml_dtypes.bfloat16

N_CORES = 8
TOKENS = 2048
D = 16384  # 128 * 128
T_CORE = TOKENS // N_CORES  # 256


def build_nc(n_tokens=T_CORE, group=32, pcopy="vector", ycopy="scalar"):
    """Build + compile the per-core program.

    group: tokens per load/store DMA (and per x/y SBUF tile).
    pcopy/ycopy: engine for the PSUM->SBUF copies of P (stage-1 out)
    and Y (stage-2 out): "vector" | "scalar" | "gpsimd".
    """
    assert n_tokens % group == 0 and group % 4 == 0

    nc = bacc.Bacc("TRN2", target_bir_lowering=False, debug=False)
    x = nc.dram_tensor("x", [128, n_tokens * 128], BF16, kind="ExternalInput")
    u = nc.dram_tensor("u", [128, 128], BF16, kind="ExternalInput")
    v = nc.dram_tensor("v", [128, 128], BF16, kind="ExternalInput")
    y = nc.dram_tensor("y", [128, n_tokens * 128], BF16, kind="ExternalOutput")

    def copy_op(which):
        eng = {"vector": nc.vector, "gpsimd": nc.gpsimd}.get(which)
        if eng is not None:
            return eng.tensor_copy
        return nc.scalar.copy

    p_copy = copy_op(pcopy)
    y_copy = copy_op(ycopy)

    with tile.TileContext(nc) as tc:
        with (
            tc.tile_pool(name="const", bufs=1) as cpool,
            tc.tile_pool(name="xin", bufs=3) as xpool,
            tc.tile_pool(name="yout", bufs=3) as ypool,
            tc.tile_pool(name="pmid", bufs=4) as ppool,
            tc.tile_pool(name="psa", bufs=4, space="PSUM") as pspool_a,
            tc.tile_pool(name="psb", bufs=4, space="PSUM") as pspool_b,
        ):
            u_sb = cpool.tile([128, 128], BF16)
            v_sb = cpool.tile([128, 128], BF16)
            nc.sync.dma_start(u_sb[:], u[:])
            nc.sync.dma_start(v_sb[:], v[:])

            gcols = group * 128
            for g in range(n_tokens // group):
                xt = xpool.tile([128, gcols], BF16)
                nc.sync.dma_start(xt[:], x[:, g * gcols : (g + 1) * gcols])
                yt = ypool.tile([128, gcols], BF16)
                for q in range(group // 4):
                    pa = pspool_a.tile([128, 512], F32)
                    for j in range(4):
                        t = (q * 4 + j) * 128
                        nc.tensor.matmul(
                            pa[:, j * 128 : (j + 1) * 128],
                            lhsT=xt[:, t : t + 128],
                            rhs=u_sb[:],
                            start=True,
                            stop=True,
                        )
                    ps = ppool.tile([128, 512], BF16)
                    p_copy(ps[:], pa[:])
                    pb = pspool_b.tile([128, 512], F32)
                    nc.tensor.matmul(
                        pb[:], lhsT=v_sb[:], rhs=ps[:], start=True, stop=True
                    )
                    y_copy(yt[:, q * 512 : (q + 1) * 512], pb[:])
                nc.scalar.dma_start(y[:, g * gcols : (g + 1) * gcols], yt[:])
    nc.compile()
    return nc


_NC_CACHE = {}


def _get_nc(n_tokens, group, pcopy, ycopy):
    key = (n_tokens, group, pcopy, ycopy)
    if key not in _NC_CACHE:
        _NC_CACHE[key] = build_nc(n_tokens, group, pcopy, ycopy)
    return _NC_CACHE[key]


def run(x, U, V, group=32, pcopy="vector", ycopy="scalar", trace=False,
        **spmd_kwargs):
    """Shard over 8 cores, run, gather. Returns (y_full, BassKernelResults)."""
    x = np.ascontiguousarray(np.asarray(x), dtype=np.float32)
    U = np.ascontiguousarray(np.asarray(U), dtype=np.float32).astype(NP_BF16)
    V = np.ascontiguousarray(np.asarray(V), dtype=np.float32).astype(NP_BF16)
    t_total = x.shape[0]
    t_core = t_total // N_CORES
    xb = x.astype(NP_BF16)

    nc = _get_nc(t_core, group, pcopy, ycopy)
    in_maps = []
    for c in range(N_CORES):
        xc = xb[c * t_core : (c + 1) * t_core].reshape(t_core, 128, 128)
        xc = np.ascontiguousarray(xc.transpose(1, 0, 2)).reshape(128, t_core * 128)
        in_maps.append({"x": xc, "u": U, "v": V})
    res = run_bass_kernel_spmd(
        nc, in_maps, list(range(N_CORES)), trace=trace, **spmd_kwargs
    )
    out = np.empty((t_total, D), dtype=np.float32)
    for c in range(N_CORES):
        yc = np.asarray(res.results[c]["y"]).reshape(128, t_core, 128)
        # [j2, t, j1] -> [t, j1, j2]
        out[c * t_core : (c + 1) * t_core] = (
            yc.transpose(1, 2, 0).reshape(t_core, D).astype(np.float32)
        )
    return out, res


def kernel(x, U, V):
    out, _ = run(x, U, V)
    return out
